# revision 9
# baseline (speedup 1.0000x reference)
"""Trainium2 Bass kernel for nn_AEAttention (B=4, N=128, FEAT=384, FFN=768, HID=192).

Math reduction: z_mask[b,i,j,:] = z[b,j,:] * (i==j), so the (B,N,N,F) autoencoder
collapses:
  preds[b,i,j,:] = AE(0) = gelu(enc_b) @ dec_w + dec_b =: c0      for i != j
  preds[b,i,i,:] = AE(z[b,i])
  dist[b,i,j]    = c0 . z[b,j] =: d0[b,j]                         for i != j
  dist[b,i,i]    = AE(z[b,i]) . z[b,i] =: d1[b,i]
Softmax row i only differs from the shared row d0 at the diagonal:
  e0[j] = exp(d0[j]), w1[i] = exp(d1[i]), S = sum_j e0[j],
  W[j,i] = e0[j] + (w1[i]-e0[i]) [j==i]          (attn weights, unnormalized)
  numT = xh^T @ W ;  out[i,:] = (numT^T V_w)[i,:] / (S - e0[i] + w1[i]) + V_b
(The max-subtraction is dropped: softmax is shift-invariant and the logits for
this problem are O(13), far from f32 exp overflow.)

LayerNorm affine (ln_w, ln_b) is folded into enc_w/enc_b/c0/dec_w on the host;
z_hat = (z - mu) * rstd is the only on-chip normalization.

The AE-hidden gelu uses the tanh approximation (inner cubic on DVE + Tanh on
ACT): tanh lives in the same activation-table set as exp, so the kernel needs
only gelu -> sqrt -> exp table loads, each overlapped with compute.

Sharding: 8 cores = (4 batches) x (2 halves of the query dim). Token order is
rolled per-core on the host (attention here is permutation-equivariant), so each
core always computes output rows 0:64 of its (rolled) batch.

DMA: 5 input loads split across the two HWDGE queues (SP + ACT), biggest /
earliest-needed first; biases ride the tail of matmul accumulation groups so
nothing waits on the small loads.
"""
import math
import sys

if '/opt/trn_rl_repo' not in sys.path:
    sys.path.insert(0, '/opt/trn_rl_repo')

import numpy as np

B, N, FEAT, FFN, ZDIM, HID = 4, 128, 384, 768, 384, 192
LN_EPS = 1e-5
NCORES = 8
OWN = 64  # output rows per core

# Compute dtype for TensorEngine operands: "bf16" (1 cyc/row) or "f32r".
COMPUTE_DT = "bf16"

# gelu_tanh(x) = t*(1+tanh(K1*t + K3*t^3)) with t = x/2
GK1 = 2.0 * 0.7978845608028654
GK3 = 8.0 * 0.044715 * 0.7978845608028654

# kaux column layout (compute dtype; matmul operands only):
#   0:128   identity
#   128:256 ones
#   256:260 c0' (3 cols + pad)
#   260     dlnb[0:128]
#   261     dlnb[128:192] (rows 0:64)
#   262:272 pad
KAUX_COLS = 272
# auxf column layout (f32; ACT bias / DVE scalar operands):
#   0 enc_b'[0:128] | 1 enc_b'[128:192] (rows 0:64) | 2:5 dec_b' |
#   5 d0 const (c0 . ln_b) | 6 d1 const (dec_b . ln_b, rows 0:64) | 7 LN_EPS
AUXF_COLS = 8

_CACHE = {}


def _patch_tile_drain(tile):
    """walrus in this container only accepts 1 sync-wait command per CTRL
    instruction; Tile's kernel-tail drain can carry many. Split the drain's
    waits over several drain instructions."""
    if getattr(tile.TileContext, '_drain_patched', False):
        return
    from concourse import mybir

    def _drain_and_barrier(self, tick_clock, wait_clock):
        nc = self.nc
        drain_inst = nc.sync.drain()
        wait_clock.add_sem_waits(
            drain_inst.ins, tile.ScopedClock({None: tick_clock.global_clock})
        )
        mi = drain_inst.ins
        waits = list(mi.sync_info.on_wait) if mi.sync_info else []
        if len(waits) > 1:
            # Keep one wait on the drain; spread the rest as one-wait NoOps
            # round-robin over all engine sequencers so they execute in
            # parallel. The all_engine_barrier below joins them, preserving
            # the original "drain waits on the whole global clock" semantics.
            mi.sync_info = mybir.SyncInfo(on_wait=waits[:1], on_update=[])
            engines = [nc.sync, nc.gpsimd, nc.scalar, nc.vector, nc.tensor]
            for i, wt_ in enumerate(waits[1:]):
                n2 = engines[i % len(engines)].nop()
                n2.ins.sync_info = mybir.SyncInfo(on_wait=[wt_], on_update=[])
        nc.all_engine_barrier()
        assert self.sems is not None
        popped = self.nc._tile_sem_poison_stack.pop()
        assert popped is self._sem_poison
        nc.clear_and_free_semaphores(list(self.sems.allocated().values()))
        nc.all_engine_barrier()

    tile.TileContext._drain_and_barrier = _drain_and_barrier
    tile.TileContext._drain_patched = True


def _split_excess_waits(nc, mybir, maxw=1):
    """This container's walrus accepts only one sync-wait command per
    instruction. Move excess waits onto InstNoOp carriers inserted just before
    the over-subscribed instruction on the same engine."""
    for fn in nc.m.functions:
        for blk in fn.blocks:
            new = []
            changed = False
            for inst in blk.instructions:
                si = inst.sync_info
                waits = list(si.on_wait) if si and si.on_wait else []
                if len(waits) > maxw:
                    changed = True
                    extra = waits[:-maxw]
                    ups = list(si.on_update) if si.on_update else []
                    inst.sync_info = mybir.SyncInfo(
                        on_wait=waits[-maxw:], on_update=ups)
                    for i in range(0, len(extra), maxw):
                        nop = mybir.InstNoOp(
                            name=nc.get_next_instruction_name(),
                            engine=inst.engine, ins=[], outs=[])
                        nop.sync_info = mybir.SyncInfo(
                            on_wait=extra[i:i + maxw], on_update=[])
                        new.append(nop)
                new.append(inst)
            if changed:
                blk.instructions = new


def _build_nc(dt_name):
    import concourse.bass as bass
    import concourse.tile as tile
    from concourse import mybir

    _patch_tile_drain(tile)

    F32 = mybir.dt.float32
    DT = {"bf16": mybir.dt.bfloat16, "f32r": mybir.dt.float32r,
          "f32": mybir.dt.float32}[dt_name]
    AF = mybir.ActivationFunctionType
    OP = mybir.AluOpType
    AX = mybir.AxisListType

    nc = bass.Bass()
    dp = nc.declare_dram_parameter
    d_xuw0 = dp("xuw0", [128, 384 + FFN], DT, isOutput=False)
    d_uw12 = dp("uw12", [128, 2 * FFN], DT, isOutput=False)
    d_kaux = dp("kaux", [128, KAUX_COLS], DT, isOutput=False)
    d_auxf = dp("auxf", [128, AUXF_COLS], F32, isOutput=False)
    d_ubvb = dp("ubvb", [1, FFN + FEAT], DT, isOutput=False)
    d_bigw = dp("bigw", [128, 3 * HID + 2 * ZDIM + 3 * FEAT], DT,
                isOutput=False)
    d_out = dp("out", [OWN, FEAT], F32, isOutput=True)

    mm = nc.tensor.matmul

    with tile.TileContext(nc) as tc:
        with tc.tile_pool(name="w", bufs=1) as w, \
             tc.tile_pool(name="ps", bufs=1, space="PSUM") as ps:

            def wt(name, p, f, dt=None):
                return w.tile([p, f], dt or DT, name=name, tag=name)

            # ---- input DMAs, split across the two HWDGE queues ----
            xuw0 = wt("xuw0", 128, 384 + FFN)
            nc.sync.dma_start(xuw0[:, :], d_xuw0[:, :])
            kaux = wt("kaux", 128, KAUX_COLS)
            nc.scalar.dma_start(kaux[:, :], d_kaux[:, :])
            uw12 = wt("uw12", 128, 2 * FFN)
            nc.sync.dma_start(uw12[:, :], d_uw12[:, :])
            auxf = wt("auxf", 128, AUXF_COLS, F32)
            nc.scalar.dma_start(auxf[:, :], d_auxf[:, :])
            ubvb = wt("ubvb", 1, FFN + FEAT)
            nc.scalar.dma_start(ubvb[:, :], d_ubvb[:, :])
            bigw = wt("bigw", 128, 3 * HID + 2 * ZDIM + 3 * FEAT)
            nc.scalar.dma_start(bigw[:, :], d_bigw[:, :])

            # ---- constant/param views ----
            ident = kaux[:, 0:128]
            ones128 = kaux[:, 128:256]
            ones2 = kaux[:, 128:130]
            ones_row = kaux[0:1, 128:256]
            c0p = kaux[:, 256:260]
            dlnb0 = kaux[:, 260:262]          # col0 = dlnb[0:128]
            dlnb1 = kaux[0:OWN, 261:263]      # col0 = dlnb[128:192]
            encb0 = auxf[:, 0:1]
            encb1 = auxf[0:OWN, 1:2]
            decb = auxf[:, 2:5]
            d0cb = auxf[:, 5:6]
            dblb = auxf[0:OWN, 6:7]
            epsc = auxf[:, 7:8]
            ub = ubvb[0:1, 0:FFN]
            vb = ubvb[0:1, FFN:FFN + FEAT]
            encw = bigw[:, 0:576]
            decw = bigw[:, 576:1344]
            vw = bigw[:, 1344:2496]

            xT = xuw0[:, 0:384]

            def uwk(k):  # U_w rows 128k:128(k+1), all 768 cols
                if k == 0:
                    return xuw0[:, 384:384 + FFN]
                return uw12[:, FFN * (k - 1):FFN * k]

            # ---- U matmul: h = gelu(x @ U_w + U_b), bias rides last ----
            ps_z = ps.tile([128, 384], F32, name="ps_z", tag="big", bufs=2)
            ps_xh = ps.tile([128, 384], F32, name="ps_xh", tag="big", bufs=2)
            for k in range(3):
                lhs = xT[:, 128 * k:128 * (k + 1)]
                u = uwk(k)
                mm(ps_z[:, :], lhs, u[:, 384:768], start=(k == 0), stop=False)
                mm(ps_xh[:, :], lhs, u[:, 0:384], start=(k == 0), stop=False)
            mm(ps_z[:, :], ones_row[0:1, 0:128], ub[0:1, 384:768],
               start=False, stop=True)
            mm(ps_xh[:, :], ones_row[0:1, 0:128], ub[0:1, 0:384],
               start=False, stop=True)

            z0 = wt("z0", 128, 384)
            nc.scalar.activation(z0[:, :], ps_z[:, :], AF.Gelu)
            xh = wt("xh", 128, 384)
            nc.scalar.activation(xh[:, :], ps_xh[:, :], AF.Gelu)

            # V_b broadcast staged for the final fused epilogue
            ps_vbb = ps.tile([OWN, 384], F32, name="ps_vbb", tag="big", bufs=2)
            mm(ps_vbb[:, :], ones_row[0:1, 0:OWN], vb[0:1, :],
               start=True, stop=True)

            # ---- LayerNorm stats, one pass: var = E[z^2] - mu^2 ----
            musum = wt("musum", 128, 1, F32)
            nc.vector.reduce_sum(musum[:, :], z0[:, :], axis=AX.X)
            sq = wt("sq", 128, 384)
            vsum = wt("vsum", 128, 1, F32)
            nc.vector.scalar_tensor_tensor(sq[:, :], z0[:, :], 1.0, z0[:, :],
                                           op0=OP.mult, op1=OP.mult,
                                           accum_out=vsum[:, 0:1])
            musq = wt("musq", 128, 1, F32)
            nc.vector.scalar_tensor_tensor(musq[:, :], musum[:, :], 1.0 / ZDIM,
                                           musum[:, :], op0=OP.mult,
                                           op1=OP.mult)
            v2 = wt("v2", 128, 1, F32)
            nc.vector.tensor_sub(v2[:, :], vsum[:, :], musq[:, :])
            std = wt("std", 128, 1, F32)
            nc.scalar.activation(std[:, :], v2[:, :], AF.Sqrt,
                                 bias=epsc[:, 0:1], scale=1.0 / ZDIM)
            rstd = wt("rstd", 128, 1, F32)
            nc.vector.reciprocal(rstd[:, :], std[:, :])
            nmurstd = wt("nmurstd", 128, 1, F32)
            nc.vector.scalar_tensor_tensor(nmurstd[:, :], musum[:, :],
                                           -1.0 / ZDIM, rstd[:, :],
                                           op0=OP.mult, op1=OP.mult)
            # z_hat = z*rstd - mu*rstd  (ln affine folded into weights)
            zn = wt("zn", 128, 384)
            nc.vector.tensor_scalar(zn[:, :], z0[:, :], rstd[:, 0:1],
                                    nmurstd[:, 0:1], op0=OP.mult, op1=OP.add)

            # ---- transpose z_hat -> zT [feat, token] ----
            zT = wt("zT", 128, 384)
            for k in range(3):
                pt = ps.tile([128, 128], DT, name="pt", tag="pt", bufs=2)
                nc.tensor.transpose(pt[:, :], zn[:, 128 * k:128 * (k + 1)],
                                    ident)
                nc.vector.tensor_copy(zT[:, 128 * k:128 * (k + 1)], pt[:, :])

            # ---- logit column d0 (all j): zT . c0' (+const via exp bias) ----
            ps_d0c = ps.tile([128, 2], F32, name="ps_d0c", tag="sm2", bufs=2)
            for k in range(3):
                mm(ps_d0c[:, :], zT[:, 128 * k:128 * (k + 1)],
                   c0p[:, k:k + 2], start=(k == 0), stop=(k == 2))

            # ---- AE enc on own 64 tokens ----
            ps_h0 = ps.tile([128, OWN], F32, name="ps_h0", tag="sm", bufs=2)
            ps_h1 = ps.tile([OWN, OWN], F32, name="ps_h1", tag="sm", bufs=2)
            for k in range(3):
                rhs = zT[:, 128 * k:128 * k + OWN]
                mm(ps_h0[:, :], encw[:, 192 * k:192 * k + 128], rhs,
                   start=(k == 0), stop=(k == 2))
                mm(ps_h1[:, :], encw[:, 192 * k + 128:192 * (k + 1)], rhs,
                   start=(k == 0), stop=(k == 2))

            # ---- softmax weights: e0 = exp(d0 + d0const) ----
            e0 = wt("e0", 128, 2)
            nc.gpsimd.tensor_copy(e0[:, 1:2], ones2[:, 0:1])
            nc.scalar.activation(e0[:, 0:1], ps_d0c[:, 0:1], AF.Exp,
                                 bias=d0cb[:, 0:1])
            ps_sbc = ps.tile([128, 2], F32, name="ps_sbc", tag="sm2", bufs=2)
            mm(ps_sbc[:, :], ones128, e0[:, 0:2], start=True, stop=True)

            # ---- AE hidden gelu via tanh approximation (exp table set) ----
            # gelu(x) ~= t*(1+tanh(GK1*t + GK3*t^3)), t = (psum + enc_b')/2.
            # Phase 1 (cubic) for both halves first so the in-order DVE never
            # stalls on the ACT tanh.
            def tg_inner(psum, bias_col, p, name):
                t1 = wt(name + "_t1", p, OWN, F32)
                nc.vector.tensor_scalar(t1[:, :], psum[:, :], bias_col, 0.5,
                                        op0=OP.add, op1=OP.mult)
                t2 = wt(name + "_t2", p, OWN, F32)
                nc.vector.scalar_tensor_tensor(t2[:, :], t1[:, :], 1.0,
                                               t1[:, :], op0=OP.mult,
                                               op1=OP.mult)
                uu = wt(name + "_u", p, OWN, F32)
                nc.vector.scalar_tensor_tensor(uu[:, :], t2[:, :], GK3,
                                               t1[:, :], op0=OP.mult,
                                               op1=OP.mult)
                inner = wt(name + "_in", p, OWN, F32)
                nc.vector.scalar_tensor_tensor(inner[:, :], t1[:, :], GK1,
                                               uu[:, :], op0=OP.mult,
                                               op1=OP.add)
                return t1, inner

            t1_0, in_0 = tg_inner(ps_h0, encb0[:, 0:1], 128, "h0")
            t1_1, in_1 = tg_inner(ps_h1, encb1[:, 0:1], OWN, "h1")
            th0 = wt("th0", 128, OWN, F32)
            nc.scalar.activation(th0[:, :], in_0[:, :], AF.Tanh)
            th1 = wt("th1", OWN, OWN, F32)
            nc.scalar.activation(th1[:, :], in_1[:, :], AF.Tanh)
            h0 = wt("h0", 128, OWN)
            nc.vector.scalar_tensor_tensor(h0[:, :], th0[:, :], 1.0,
                                           t1_0[:, :], op0=OP.add,
                                           op1=OP.mult)
            h1 = wt("h1", OWN, OWN)
            nc.vector.scalar_tensor_tensor(h1[:, :], th1[:, :], 1.0,
                                           t1_1[:, :], op0=OP.add,
                                           op1=OP.mult)

            # ---- AE dec + P = (dec_out + dec_b') * z_hat ----
            Pt = wt("Pt", 128, 3 * OWN)
            for k in range(3):
                ps_d = ps.tile([128, OWN], F32, name="ps_d", tag="pt", bufs=2)
                mm(ps_d[:, :], decw[0:128, 128 * k:128 * (k + 1)], h0[:, :],
                   start=True, stop=False)
                mm(ps_d[:, :], decw[0:OWN, 384 + 128 * k:384 + 128 * (k + 1)],
                   h1[:, :], start=False, stop=True)
                nc.vector.scalar_tensor_tensor(
                    Pt[:, OWN * k:OWN * (k + 1)], ps_d[:, :], decb[:, k:k + 1],
                    zT[:, 128 * k:128 * k + OWN], op0=OP.add, op1=OP.mult)

            # ---- diag logit d1 = sum_f P + h . dlnb (+const via exp bias) ----
            ps_d1c = ps.tile([OWN, 2], F32, name="ps_d1c", tag="sm2", bufs=2)
            for k in range(3):
                mm(ps_d1c[:, :], Pt[:, OWN * k:OWN * (k + 1)],
                   ones2, start=(k == 0), stop=False)
            mm(ps_d1c[:, :], h0[:, :], dlnb0, start=False, stop=False)
            mm(ps_d1c[:, :], h1[:, :], dlnb1, start=False, stop=True)
            w1 = wt("w1", OWN, 1, F32)
            nc.scalar.activation(w1[:, 0:1], ps_d1c[:, 0:1], AF.Exp,
                                 bias=dblb[:, 0:1])

            # ---- attn weight matrix W[j,i] = e0[j] + delta[i]*(j==i) ----
            delta = wt("delta", OWN, 1, F32)
            nc.vector.tensor_sub(delta[:, :], w1[:, :], e0[0:OWN, 0:1])
            ddiag = wt("ddiag", OWN, OWN)
            nc.vector.tensor_scalar(ddiag[:, :], ident[0:OWN, 0:OWN],
                                    delta[:, 0:1], None, op0=OP.mult)
            Wm = wt("Wm", 128, OWN)
            nc.gpsimd.tensor_copy(Wm[:, :],
                                  e0[:, 0:1].broadcast_to([128, OWN]))
            nc.gpsimd.tensor_tensor(Wm[0:OWN, :], Wm[0:OWN, :], ddiag[:, :],
                                    op=OP.add)
            denom = wt("denom", OWN, 1, F32)
            nc.vector.tensor_add(denom[:, :], delta[:, :], ps_sbc[0:OWN, 0:1])
            rden = wt("rden", OWN, 1, F32)
            nc.vector.reciprocal(rden[:, :], denom[:, :])
            vbbc = wt("vbbc", OWN, 384, F32)
            nc.vector.tensor_copy(vbbc[:, :], ps_vbb[:, :])

            # ---- numT[f,i] = sum_j xh[j,f] W[j,i] (feat-major, no transpose) ----
            numT = wt("numT", 128, 3 * OWN)
            for k in range(3):
                ps_nt = ps.tile([128, OWN], F32, name="ps_nt", tag="pt",
                                bufs=2)
                mm(ps_nt[:, :], xh[:, 128 * k:128 * (k + 1)], Wm[:, :],
                   start=True, stop=True)
                nc.vector.tensor_copy(numT[:, OWN * k:OWN * (k + 1)],
                                      ps_nt[:, :])

            # ---- res = (numT^T @ V_w) * rden + vb ----
            ps_res = ps.tile([OWN, 384], F32, name="ps_res", tag="big",
                             bufs=2)
            for k in range(3):
                mm(ps_res[:, :], numT[:, OWN * k:OWN * (k + 1)],
                   vw[:, 384 * k:384 * (k + 1)], start=(k == 0), stop=(k == 2))
            res = wt("res", OWN, 384, F32)
            nc.vector.scalar_tensor_tensor(
                res[:, :], ps_res[:, :], rden[:, 0:1], vbbc[:, :],
                op0=OP.mult, op1=OP.add)
            nc.sync.dma_start(d_out[:, :], res[:, :])

    _split_excess_waits(nc, mybir)
    return nc


def _gelu64(x):
    x = np.asarray(x, dtype=np.float64)
    erf = np.vectorize(math.erf)
    return x * 0.5 * (1.0 + erf(x / math.sqrt(2.0)))


def _np_dt(dt_name):
    if dt_name == "bf16":
        import ml_dtypes
        return ml_dtypes.bfloat16
    return np.float32


def _prep_weights(U_w, U_b, ln_w, ln_b, enc_w, enc_b, dec_w, dec_b, V_w, V_b,
                  dt_name=None):
    dt_name = dt_name or COMPUTE_DT
    ndt = _np_dt(dt_name)
    f32 = lambda a: np.ascontiguousarray(np.asarray(a, dtype=np.float32))
    cvt = lambda a: np.ascontiguousarray(np.asarray(a).astype(ndt))
    lnw, lnb = f32(ln_w), f32(ln_b)

    uw = f32(U_w).reshape(3, 128, FFN).transpose(1, 0, 2)
    uw0 = uw[:, 0, :]
    uw12 = uw[:, 1:, :].reshape(128, 2 * FFN)

    encw_f = lnw[:, None] * f32(enc_w)
    encb_f = f32(enc_b) + lnb @ f32(enc_w)
    decw_f = f32(dec_w) * lnw[None, :]
    decb_f = f32(dec_b) * lnw
    dlnb = f32(dec_w) @ lnb
    dbl = float(f32(dec_b) @ lnb)
    # c0 = gelu(enc_b) @ dec_w + dec_b  (weight-only constant, float64)
    c0 = (_gelu64(enc_b) @ np.asarray(dec_w, np.float64)
          + np.asarray(dec_b, np.float64)).astype(np.float32)
    c0p = c0 * lnw
    d0c = float(c0 @ lnb)

    encw = encw_f.reshape(3, 128, HID).transpose(1, 0, 2).reshape(128, 3 * HID)
    vwf = f32(V_w).reshape(3, 128, FEAT).transpose(1, 0, 2).reshape(128, 3 * FEAT)
    decw = np.zeros((128, 2 * ZDIM), np.float32)
    decw[:, :ZDIM] = decw_f[0:128, :]
    decw[:OWN, ZDIM:] = decw_f[128:192, :]
    bigw = np.concatenate([encw, decw, vwf], axis=1)

    kaux = np.zeros((128, KAUX_COLS), np.float32)
    kaux[:, 0:128] = np.eye(128, dtype=np.float32)
    kaux[:, 128:256] = 1.0
    kaux[:, 256:259] = c0p.reshape(3, 128).T
    kaux[:, 260] = dlnb[0:128]
    kaux[:OWN, 261] = dlnb[128:192]

    auxf = np.zeros((128, AUXF_COLS), np.float32)
    auxf[:, 0] = encb_f[0:128]
    auxf[:OWN, 1] = encb_f[128:192]
    auxf[:, 2:5] = decb_f.reshape(3, 128).T
    auxf[:, 5] = d0c
    auxf[:OWN, 6] = dbl
    auxf[:, 7] = LN_EPS

    ubvb = np.concatenate([f32(U_b), f32(V_b)]).reshape(1, FFN + FEAT)
    return {
        "uw0": cvt(uw0),
        "uw12": cvt(uw12),
        "kaux": cvt(kaux),
        "auxf": auxf,
        "ubvb": cvt(ubvb),
        "bigw": cvt(bigw),
    }


def _get_nc(dt_name=None):
    dt_name = dt_name or COMPUTE_DT
    key = ("nc", dt_name)
    if key not in _CACHE:
        _CACHE[key] = _build_nc(dt_name)
    return _CACHE[key]


def make_in_maps(x, weights, dt_name=None):
    dt_name = dt_name or COMPUTE_DT
    ndt = _np_dt(dt_name)
    x = np.asarray(x).astype(ndt)
    shared = {k: weights[k] for k in ("uw12", "kaux", "auxf", "ubvb", "bigw")}
    in_maps = []
    for c in range(NCORES):
        b, ih = divmod(c, 2)
        xs = np.roll(x[b], -OWN * ih, axis=0)
        # pre-transpose: [128, 3*128] where chunk k = x[:, 128k:128(k+1)].T
        xt = xs.T.reshape(3, 128, 128).transpose(1, 0, 2).reshape(128, 384)
        xuw0 = np.ascontiguousarray(
            np.concatenate([xt, weights["uw0"]], axis=1))
        in_maps.append({"xuw0": xuw0, **shared})
    return in_maps


def assemble(results):
    out = np.empty((B, N, FEAT), np.float32)
    for c in range(NCORES):
        b, ih = divmod(c, 2)
        out[b, OWN * ih:OWN * (ih + 1), :] = results[c]["out"]
    return out


def kernel(x, U_w, U_b, ln_w, ln_b, enc_w, enc_b, dec_w, dec_b, V_w, V_b):
    from concourse.bass_utils import run_bass_kernel_spmd
    nc = _get_nc()
    weights = _prep_weights(U_w, U_b, ln_w, ln_b, enc_w, enc_b, dec_w, dec_b,
                            V_w, V_b)
    in_maps = make_in_maps(x, weights)
    r = run_bass_kernel_spmd(nc, in_maps, core_ids=list(range(NCORES)))
    return assemble(r.results)


# revision 16
# speedup vs baseline: 1.0050x; 1.0050x over previous
"""Trainium2 Bass kernel for nn_AEAttention (B=4, N=128, FEAT=384, FFN=768, HID=192).

Math reduction: z_mask[b,i,j,:] = z[b,j,:] * (i==j), so the (B,N,N,F) autoencoder
collapses:
  preds[b,i,j,:] = AE(0) = gelu(enc_b) @ dec_w + dec_b =: c0      for i != j
  preds[b,i,i,:] = AE(z[b,i])
  dist[b,i,j]    = c0 . z[b,j] =: d0[b,j]                         for i != j
  dist[b,i,i]    = AE(z[b,i]) . z[b,i] =: d1[b,i]
Softmax row i only differs from the shared row d0 at the diagonal, and the
output projection is linear, so with XV := xh @ V_w (all tokens):
  e0[j] = exp(d0[j]), w1[i] = exp(d1[i]), S = sum_j e0[j], delta = w1 - e0
  out[i,:] = (sum_j e0[j] XV[j,:] + delta[i] XV[i,:]) / (S - e0[i] + w1[i]) + V_b
XV is computed early (only needs xh and V_w), so after the scalar softmax
pieces the output is just two fused DVE ops.
(The max-subtraction is dropped: softmax is shift-invariant and the logits for
this problem are O(13), far from f32 exp overflow.)

LayerNorm affine (ln_w, ln_b) is folded into enc_w/enc_b/c0/dec_w on the host;
z_hat = (z - mu) * rstd is the only on-chip normalization, and the rstd scale
rides the z-transpose as a diagonal right-operand (transpose out = zc^T @
diag(rstd)).

Activation-table plan: Gelu (z half) -> Sqrt -> everything else (tanh + exp
live in the same exp_and_others set): xh-gelu and the AE-hidden gelu use the
tanh approximation (cubic on DVE, Tanh on ACT), so only two mid-kernel table
loads occur and both overlap compute.

Sharding: 8 cores = (4 batches) x (2 halves of the query dim). Token order is
rolled per-core on the host (attention here is permutation-equivariant), so each
core always computes output rows 0:64 of its (rolled) batch.

DMA: the z-half of U_w ships with x in the first load so the z matmul starts
as early as possible; triggers are split across the two HWDGE queues (SP+ACT).
"""
import math
import sys

if '/opt/trn_rl_repo' not in sys.path:
    sys.path.insert(0, '/opt/trn_rl_repo')

import numpy as np

B, N, FEAT, FFN, ZDIM, HID = 4, 128, 384, 768, 384, 192
LN_EPS = 1e-5
NCORES = 8
OWN = 64  # output rows per core

COMPUTE_DT = "bf16"

# gelu_tanh(x) = t*(1+tanh(GK1*t + GK3*t^3)) with t = x/2
GK1 = 2.0 * 0.7978845608028654
GK3 = 8.0 * 0.044715 * 0.7978845608028654

# kaux columns (compute dtype; matmul operands only):
#   0:128 identity | 128:256 ones | 256:260 c0' (3 cols + pad) |
#   260 dlnb[0:128] | 261 dlnb[128:192] (rows 0:64) | 262:272 pad
KAUX_COLS = 272
# auxf columns (f32): 0 enc_b'[0:128] | 1 enc_b'[128:192] (rows 0:64) |
#   2:5 dec_b' | 5 d0 const | 6 d1 const (rows 0:64) | 7 pad |
#   8:392 V_b broadcast (rows 0:64)
AUXF_COLS = 8 + FEAT

_CACHE = {}


def _patch_tile_drain(tile):
    """walrus in this container only accepts 1 sync-wait command per CTRL
    instruction; Tile's kernel-tail drain can carry many. Split the drain's
    waits over several drain instructions."""
    if getattr(tile.TileContext, '_drain_patched', False):
        return
    from concourse import mybir

    def _drain_and_barrier(self, tick_clock, wait_clock):
        nc = self.nc
        drain_inst = nc.sync.drain()
        wait_clock.add_sem_waits(
            drain_inst.ins, tile.ScopedClock({None: tick_clock.global_clock})
        )
        mi = drain_inst.ins
        waits = list(mi.sync_info.on_wait) if mi.sync_info else []
        if len(waits) > 1:
            mi.sync_info = mybir.SyncInfo(on_wait=waits[:1], on_update=[])
            engines = [nc.sync, nc.gpsimd, nc.scalar, nc.vector, nc.tensor]
            for i, wt_ in enumerate(waits[1:]):
                n2 = engines[i % len(engines)].nop()
                n2.ins.sync_info = mybir.SyncInfo(on_wait=[wt_], on_update=[])
        nc.all_engine_barrier()
        assert self.sems is not None
        popped = self.nc._tile_sem_poison_stack.pop()
        assert popped is self._sem_poison
        nc.clear_and_free_semaphores(list(self.sems.allocated().values()))
        nc.all_engine_barrier()

    tile.TileContext._drain_and_barrier = _drain_and_barrier
    tile.TileContext._drain_patched = True


def _split_excess_waits(nc, mybir, maxw=1):
    """This container's walrus accepts only one sync-wait command per
    instruction. Move excess waits onto InstNoOp carriers inserted just before
    the over-subscribed instruction on the same engine."""
    for fn in nc.m.functions:
        for blk in fn.blocks:
            new = []
            changed = False
            for inst in blk.instructions:
                si = inst.sync_info
                waits = list(si.on_wait) if si and si.on_wait else []
                if len(waits) > maxw:
                    changed = True
                    extra = waits[:-maxw]
                    ups = list(si.on_update) if si.on_update else []
                    inst.sync_info = mybir.SyncInfo(
                        on_wait=waits[-maxw:], on_update=ups)
                    for i in range(0, len(extra), maxw):
                        nop = mybir.InstNoOp(
                            name=nc.get_next_instruction_name(),
                            engine=inst.engine, ins=[], outs=[])
                        nop.sync_info = mybir.SyncInfo(
                            on_wait=extra[i:i + maxw], on_update=[])
                        new.append(nop)
                new.append(inst)
            if changed:
                blk.instructions = new


def _build_nc(dt_name):
    import concourse.bass as bass
    import concourse.tile as tile
    from concourse import mybir

    _patch_tile_drain(tile)

    F32 = mybir.dt.float32
    DT = {"bf16": mybir.dt.bfloat16, "f32r": mybir.dt.float32r,
          "f32": mybir.dt.float32}[dt_name]
    AF = mybir.ActivationFunctionType
    OP = mybir.AluOpType
    AX = mybir.AxisListType

    nc = bass.Bass()
    dp = nc.declare_dram_parameter
    d_xuwz = dp("xuwz", [128, 384 + 3 * 384], DT, isOutput=False)
    d_uwxh = dp("uwxh", [128, 3 * 384], DT, isOutput=False)
    d_kaux = dp("kaux", [128, KAUX_COLS], DT, isOutput=False)
    d_auxf = dp("auxf", [128, AUXF_COLS], F32, isOutput=False)
    d_ubvb = dp("ubvb", [1, FFN + FEAT], DT, isOutput=False)
    d_bigw = dp("bigw", [128, 3 * HID + 2 * ZDIM + 3 * FEAT], DT,
                isOutput=False)
    d_out = dp("out", [OWN, FEAT], F32, isOutput=True)

    mm = nc.tensor.matmul

    with tile.TileContext(nc) as tc:
        with tc.tile_pool(name="w", bufs=1) as w, \
             tc.tile_pool(name="ps", bufs=1, space="PSUM") as ps:

            def wt(name, p, f, dt=None):
                return w.tile([p, f], dt or DT, name=name, tag=name)

            # ---- input DMAs, split across the two HWDGE queues ----
            xuwz = wt("xuwz", 128, 384 + 3 * 384)
            nc.sync.dma_start(xuwz[:, :], d_xuwz[:, :])
            kaux = wt("kaux", 128, KAUX_COLS)
            nc.scalar.dma_start(kaux[:, :], d_kaux[:, :])
            uwxh = wt("uwxh", 128, 3 * 384)
            nc.sync.dma_start(uwxh[:, :], d_uwxh[:, :])
            ubvb = wt("ubvb", 1, FFN + FEAT)
            nc.scalar.dma_start(ubvb[:, :], d_ubvb[:, :])
            auxf = wt("auxf", 128, AUXF_COLS, F32)
            nc.scalar.dma_start(auxf[:, :], d_auxf[:, :])
            bigw = wt("bigw", 128, 3 * HID + 2 * ZDIM + 3 * FEAT)
            nc.scalar.dma_start(bigw[:, :], d_bigw[:, :])

            # ---- views ----
            ident = kaux[:, 0:128]
            ones2 = kaux[:, 128:130]
            ones128 = kaux[:, 128:256]
            ones_row = kaux[0:1, 128:256]
            c0p = kaux[:, 256:260]
            dlnb0 = kaux[:, 260:262]
            dlnb1 = kaux[0:OWN, 261:263]
            encb0 = auxf[:, 0:1]
            encb1 = auxf[0:OWN, 1:2]
            decb = auxf[:, 2:5]
            d0cb = auxf[:, 5:6]
            dblb = auxf[0:OWN, 6:7]
            vbbc = auxf[0:OWN, 8:8 + FEAT]
            ub = ubvb[0:1, 0:FFN]
            encw = bigw[:, 0:576]
            decw = bigw[:, 576:1344]
            vw = bigw[:, 1344:2496]
            xT = xuwz[:, 0:384]

            # ---- U matmul, z half first (bias rides first) ----
            ps_z = ps.tile([128, 384], F32, name="ps_z", tag="big", bufs=2)
            mm(ps_z[:, :], ones_row[0:1, 0:128], ub[0:1, 384:768],
               start=True, stop=False)
            for k in range(3):
                mm(ps_z[:, :], xT[:, 128 * k:128 * (k + 1)],
                   xuwz[:, 384 * (k + 1):384 * (k + 2)],
                   start=False, stop=(k == 2))
            z0 = wt("z0", 128, 384)
            nc.scalar.activation(z0[:, :], ps_z[:, :], AF.Gelu)

            ps_xh = ps.tile([128, 384], F32, name="ps_xh", tag="big", bufs=2)
            mm(ps_xh[:, :], ones_row[0:1, 0:128], ub[0:1, 0:384],
               start=True, stop=False)
            for k in range(3):
                mm(ps_xh[:, :], xT[:, 128 * k:128 * (k + 1)],
                   uwxh[:, 384 * k:384 * (k + 1)],
                   start=False, stop=(k == 2))

            # ---- LayerNorm stats (one pass) + centered z ----
            epsc = wt("epsc", 128, 1, F32)
            nc.vector.memset(epsc[:, :], LN_EPS)
            musum = wt("musum", 128, 1, F32)
            nc.vector.reduce_sum(musum[:, :], z0[:, :], axis=AX.X)
            sq = wt("sq", 128, 384)
            vsum = wt("vsum", 128, 1, F32)
            nc.vector.scalar_tensor_tensor(sq[:, :], z0[:, :], 1.0, z0[:, :],
                                           op0=OP.mult, op1=OP.mult,
                                           accum_out=vsum[:, 0:1])
            musq = wt("musq", 128, 1, F32)
            nc.vector.scalar_tensor_tensor(musq[:, :], musum[:, :], 1.0 / ZDIM,
                                           musum[:, :], op0=OP.mult,
                                           op1=OP.mult)
            v2 = wt("v2", 128, 1, F32)
            nc.vector.tensor_sub(v2[:, :], vsum[:, :], musq[:, :])
            negmu = wt("negmu", 128, 1, F32)
            nc.vector.tensor_scalar_mul(negmu[:, :], musum[:, :], -1.0 / ZDIM)
            zc = wt("zc", 128, 384)
            nc.vector.tensor_scalar_add(zc[:, :], z0[:, :], negmu[:, 0:1])
            std = wt("std", 128, 1, F32)
            nc.scalar.activation(std[:, :], v2[:, :], AF.Sqrt,
                                 bias=epsc[:, 0:1], scale=1.0 / ZDIM)
            rstd = wt("rstd", 128, 1, F32)
            nc.vector.reciprocal(rstd[:, :], std[:, :])
            # z_hat = zc * rstd (per-token scale; zc was centered early)
            zn = wt("zn", 128, 384)
            nc.vector.tensor_scalar(zn[:, :], zc[:, :], rstd[:, 0:1],
                                    None, op0=OP.mult)

            # ---- xh gelu via tanh approx (keeps ACT in the exp set) ----
            xt1 = wt("xt1", 128, 384, F32)
            nc.vector.tensor_scalar(xt1[:, :], ps_xh[:, :], 0.5, None,
                                    op0=OP.mult)
            xq = wt("xq", 128, 384, F32)
            nc.vector.scalar_tensor_tensor(xq[:, :], xt1[:, :], GK3,
                                           xt1[:, :], op0=OP.mult,
                                           op1=OP.mult)
            xin = wt("xin", 128, 384, F32)
            nc.vector.scalar_tensor_tensor(xin[:, :], xq[:, :], GK1,
                                           xt1[:, :],
                                           op0=OP.add, op1=OP.mult)
            xth = wt("xth", 128, 384, F32)
            nc.scalar.activation(xth[:, :], xin[:, :], AF.Tanh)
            xh = wt("xh", 128, 384)
            nc.vector.scalar_tensor_tensor(xh[:, :], xth[:, :], 1.0,
                                           xt1[:, :], op0=OP.add,
                                           op1=OP.mult)

            # ---- transpose z_hat -> zT [feat, token] ----
            zT = wt("zT", 128, 384)
            for k in range(3):
                pt = ps.tile([128, 128], DT, name="pt", tag="pt", bufs=2)
                nc.tensor.transpose(pt[:, :], zn[:, 128 * k:128 * (k + 1)],
                                    ident)
                nc.vector.tensor_copy(zT[:, 128 * k:128 * (k + 1)], pt[:, :])

            # ---- logit column d0 ----
            ps_d0c = ps.tile([128, 2], F32, name="ps_d0c", tag="sm2", bufs=2)
            for k in range(3):
                mm(ps_d0c[:, :], zT[:, 128 * k:128 * (k + 1)],
                   c0p[:, k:k + 2], start=(k == 0), stop=(k == 2))

            # ---- AE enc ----
            ps_h0 = ps.tile([128, OWN], F32, name="ps_h0", tag="sm", bufs=2)
            ps_h1 = ps.tile([OWN, OWN], F32, name="ps_h1", tag="sm", bufs=2)
            for k in range(3):
                rhs = zT[:, 128 * k:128 * k + OWN]
                mm(ps_h0[:, :], encw[:, 192 * k:192 * k + 128], rhs,
                   start=(k == 0), stop=(k == 2))
                mm(ps_h1[:, :], encw[:, 192 * k + 128:192 * (k + 1)], rhs,
                   start=(k == 0), stop=(k == 2))

            # ---- AE hidden gelu (tanh approx; cubic on DVE, Tanh on ACT) ----
            def tg_inner(psum, bias_col, p, name):
                t1 = wt(name + "_t1", p, OWN, F32)
                nc.vector.tensor_scalar(t1[:, :], psum[:, :], bias_col, 0.5,
                                        op0=OP.add, op1=OP.mult)
                q = wt(name + "_q", p, OWN, F32)
                nc.vector.scalar_tensor_tensor(q[:, :], t1[:, :], GK3,
                                               t1[:, :], op0=OP.mult,
                                               op1=OP.mult)
                inner = wt(name + "_in", p, OWN, F32)
                nc.vector.scalar_tensor_tensor(inner[:, :], q[:, :],
                                               GK1, t1[:, :],
                                               op0=OP.add, op1=OP.mult)
                return t1, inner

            t1_0, in_0 = tg_inner(ps_h0, encb0[:, 0:1], 128, "h0")
            t1_1, in_1 = tg_inner(ps_h1, encb1[:, 0:1], OWN, "h1")
            th0 = wt("th0", 128, OWN, F32)
            nc.scalar.activation(th0[:, :], in_0[:, :], AF.Tanh)
            th1 = wt("th1", OWN, OWN, F32)
            nc.scalar.activation(th1[:, :], in_1[:, :], AF.Tanh)
            h0 = wt("h0", 128, OWN)
            nc.vector.scalar_tensor_tensor(h0[:, :], th0[:, :], 1.0,
                                           t1_0[:, :], op0=OP.add,
                                           op1=OP.mult)
            h1 = wt("h1", OWN, OWN)
            nc.vector.scalar_tensor_tensor(h1[:, :], th1[:, :], 1.0,
                                           t1_1[:, :], op0=OP.add,
                                           op1=OP.mult)

            # ---- e0 = exp(d0 + d0const) (slack; after tanh ops on ACT) ----
            e0 = wt("e0", 128, 2)
            nc.gpsimd.tensor_copy(e0[:, 1:2], ones2[:, 0:1])
            nc.scalar.activation(e0[:, 0:1], ps_d0c[:, 0:1], AF.Exp,
                                 bias=d0cb[:, 0:1])

            # ---- slack pipeline: xh^T, XV = xh @ V_w, e0-weighted sum ----
            xhT = wt("xhT", 128, 384)
            for k in range(3):
                pt2 = ps.tile([128, 128], DT, name="pt2", tag="pt", bufs=2)
                nc.tensor.transpose(pt2[:, :], xh[:, 128 * k:128 * (k + 1)],
                                    ident)
                nc.vector.tensor_copy(xhT[:, 128 * k:128 * (k + 1)],
                                      pt2[:, :])
            ps_xv = ps.tile([128, 384], F32, name="ps_xv", tag="big", bufs=2)
            for k in range(3):
                mm(ps_xv[:, :], xhT[:, 128 * k:128 * (k + 1)],
                   vw[:, 384 * k:384 * (k + 1)], start=(k == 0), stop=(k == 2))
            xv = wt("xv", 128, 384)
            nc.vector.tensor_copy(xv[:, :], ps_xv[:, :])
            e0bc = wt("e0bc", 128, OWN)
            nc.gpsimd.tensor_copy(e0bc[:, :],
                                  e0[:, 0:1].broadcast_to([128, OWN]))
            ps_tbv = ps.tile([OWN, 384], F32, name="ps_tbv", tag="big",
                             bufs=2)
            mm(ps_tbv[:, :], e0bc[:, :], xv[:, :], start=True, stop=True)
            ps_sbc = ps.tile([128, 2], F32, name="ps_sbc", tag="sm2", bufs=2)
            mm(ps_sbc[:, :], ones128, e0[:, 0:2], start=True, stop=True)

            # ---- AE dec + P = (dec_out + dec_b') * z_hat ----
            Pt = wt("Pt", 128, 3 * OWN)
            for k in range(3):
                ps_d = ps.tile([128, OWN], F32, name="ps_d", tag="pt", bufs=2)
                mm(ps_d[:, :], decw[0:128, 128 * k:128 * (k + 1)], h0[:, :],
                   start=True, stop=False)
                mm(ps_d[:, :], decw[0:OWN, 384 + 128 * k:384 + 128 * (k + 1)],
                   h1[:, :], start=False, stop=True)
                nc.vector.scalar_tensor_tensor(
                    Pt[:, OWN * k:OWN * (k + 1)], ps_d[:, :], decb[:, k:k + 1],
                    zT[:, 128 * k:128 * k + OWN], op0=OP.add, op1=OP.mult)

            # ---- diag logit d1 ----
            ps_d1c = ps.tile([OWN, 2], F32, name="ps_d1c", tag="sm2", bufs=2)
            for k in range(3):
                mm(ps_d1c[:, :], Pt[:, OWN * k:OWN * (k + 1)],
                   ones2, start=(k == 0), stop=False)
            mm(ps_d1c[:, :], h0[:, :], dlnb0, start=False, stop=False)
            mm(ps_d1c[:, :], h1[:, :], dlnb1, start=False, stop=True)
            w1 = wt("w1", OWN, 1, F32)
            nc.scalar.activation(w1[:, 0:1], ps_d1c[:, 0:1], AF.Exp,
                                 bias=dblb[:, 0:1])

            # ---- softmax scalars ----
            delta = wt("delta", OWN, 1, F32)
            nc.vector.tensor_sub(delta[:, :], w1[:, :], e0[0:OWN, 0:1])
            rden = wt("rden", OWN, 1, F32)
            denom = wt("denom", OWN, 1, F32)

            # ---- epilogue: res = rden*TBV + (delta*rden)*XV_own + vb ----
            nc.vector.tensor_add(denom[:, :], delta[:, :], ps_sbc[0:OWN, 0:1])
            nc.vector.reciprocal(rden[:, :], denom[:, :])
            drd = wt("drd", OWN, 1, F32)
            nc.vector.scalar_tensor_tensor(drd[:, :], delta[:, :], 1.0,
                                           rden[:, :], op0=OP.mult,
                                           op1=OP.mult)
            tmid = wt("tmid", OWN, 384, F32)
            nc.vector.scalar_tensor_tensor(tmid[:, :], xv[0:OWN, :],
                                           drd[:, 0:1], vbbc[:, :],
                                           op0=OP.mult, op1=OP.add)
            res = wt("res", OWN, 384, F32)
            nc.vector.scalar_tensor_tensor(res[:, :], ps_tbv[:, :],
                                           rden[:, 0:1], tmid[:, :],
                                           op0=OP.mult, op1=OP.add)
            nc.sync.dma_start(d_out[:, :], res[:, :])

    _split_excess_waits(nc, mybir)
    return nc


def _gelu64(x):
    x = np.asarray(x, dtype=np.float64)
    erf = np.vectorize(math.erf)
    return x * 0.5 * (1.0 + erf(x / math.sqrt(2.0)))


def _np_dt(dt_name):
    if dt_name == "bf16":
        import ml_dtypes
        return ml_dtypes.bfloat16
    return np.float32


def _prep_weights(U_w, U_b, ln_w, ln_b, enc_w, enc_b, dec_w, dec_b, V_w, V_b,
                  dt_name=None):
    dt_name = dt_name or COMPUTE_DT
    ndt = _np_dt(dt_name)
    f32 = lambda a: np.ascontiguousarray(np.asarray(a, dtype=np.float32))
    cvt = lambda a: np.ascontiguousarray(np.asarray(a).astype(ndt))
    lnw, lnb = f32(ln_w), f32(ln_b)

    # uw chunks: chunk k = U_w[128k:128(k+1), :]; z half = cols 384:768
    uw = f32(U_w).reshape(3, 128, FFN).transpose(1, 0, 2)  # [128, 3, 768]
    uwz = uw[:, :, 384:768].reshape(128, 3 * 384)
    uwxh = uw[:, :, 0:384].reshape(128, 3 * 384)

    encw_f = lnw[:, None] * f32(enc_w)
    encb_f = f32(enc_b) + lnb @ f32(enc_w)
    decw_f = f32(dec_w) * lnw[None, :]
    decb_f = f32(dec_b) * lnw
    dlnb = f32(dec_w) @ lnb
    dbl = float(f32(dec_b) @ lnb)
    c0 = (_gelu64(enc_b) @ np.asarray(dec_w, np.float64)
          + np.asarray(dec_b, np.float64)).astype(np.float32)
    c0p = c0 * lnw
    d0c = float(c0 @ lnb)

    encw = encw_f.reshape(3, 128, HID).transpose(1, 0, 2).reshape(128, 3 * HID)
    vwf = f32(V_w).reshape(3, 128, FEAT).transpose(1, 0, 2).reshape(128, 3 * FEAT)
    decw = np.zeros((128, 2 * ZDIM), np.float32)
    decw[:, :ZDIM] = decw_f[0:128, :]
    decw[:OWN, ZDIM:] = decw_f[128:192, :]
    bigw = np.concatenate([encw, decw, vwf], axis=1)

    kaux = np.zeros((128, KAUX_COLS), np.float32)
    kaux[:, 0:128] = np.eye(128, dtype=np.float32)
    kaux[:, 128:256] = 1.0
    kaux[:, 256:259] = c0p.reshape(3, 128).T
    kaux[:, 260] = dlnb[0:128]
    kaux[:OWN, 261] = dlnb[128:192]

    auxf = np.zeros((128, AUXF_COLS), np.float32)
    auxf[:, 0] = encb_f[0:128]
    auxf[:OWN, 1] = encb_f[128:192]
    auxf[:, 2:5] = decb_f.reshape(3, 128).T
    auxf[:, 5] = d0c
    auxf[:OWN, 6] = dbl
    auxf[:OWN, 8:8 + FEAT] = f32(V_b)[None, :]

    ubvb = np.concatenate([f32(U_b), f32(V_b)]).reshape(1, FFN + FEAT)
    return {
        "uwz": cvt(uwz),
        "uwxh": cvt(uwxh),
        "kaux": cvt(kaux),
        "auxf": auxf,
        "ubvb": cvt(ubvb),
        "bigw": cvt(bigw),
    }


def _get_nc(dt_name=None):
    dt_name = dt_name or COMPUTE_DT
    key = ("nc", dt_name)
    if key not in _CACHE:
        _CACHE[key] = _build_nc(dt_name)
    return _CACHE[key]


def make_in_maps(x, weights, dt_name=None):
    dt_name = dt_name or COMPUTE_DT
    ndt = _np_dt(dt_name)
    x = np.asarray(x).astype(ndt)
    shared = {k: weights[k] for k in ("uwxh", "kaux", "auxf", "ubvb", "bigw")}
    in_maps = []
    for c in range(NCORES):
        b, ih = divmod(c, 2)
        xs = np.roll(x[b], -OWN * ih, axis=0)
        # pre-transpose: [128, 3*128] where chunk k = x[:, 128k:128(k+1)].T
        xt = xs.T.reshape(3, 128, 128).transpose(1, 0, 2).reshape(128, 384)
        xuwz = np.ascontiguousarray(
            np.concatenate([xt, weights["uwz"]], axis=1))
        in_maps.append({"xuwz": xuwz, **shared})
    return in_maps


def assemble(results):
    out = np.empty((B, N, FEAT), np.float32)
    for c in range(NCORES):
        b, ih = divmod(c, 2)
        out[b, OWN * ih:OWN * (ih + 1), :] = results[c]["out"]
    return out


def kernel(x, U_w, U_b, ln_w, ln_b, enc_w, enc_b, dec_w, dec_b, V_w, V_b):
    from concourse.bass_utils import run_bass_kernel_spmd
    nc = _get_nc()
    weights = _prep_weights(U_w, U_b, ln_w, ln_b, enc_w, enc_b, dec_w, dec_b,
                            V_w, V_b)
    in_maps = make_in_maps(x, weights)
    r = run_bass_kernel_spmd(nc, in_maps, core_ids=list(range(NCORES)))
    return assemble(r.results)


# revision 23
# speedup vs baseline: 1.0751x; 1.0697x over previous
"""Trainium2 Bass kernel for nn_AEAttention (B=4, N=128, FEAT=384, FFN=768, HID=192).

Math reduction: z_mask[b,i,j,:] = z[b,j,:] * (i==j), so the (B,N,N,F) autoencoder
collapses:
  preds[b,i,j,:] = AE(0) = gelu(enc_b) @ dec_w + dec_b =: c0      for i != j
  preds[b,i,i,:] = AE(z[b,i])
  dist[b,i,j]    = c0 . z[b,j] =: d0[b,j]                         for i != j
  dist[b,i,i]    = AE(z[b,i]) . z[b,i] =: d1[b,i]
Softmax row i only differs from the shared row d0 at the diagonal, and the
output projection is linear, so with XV := xh @ V_w (all tokens):
  e0[j] = exp(d0[j]), w1[i] = exp(d1[i]), S = sum_j e0[j], delta = w1 - e0
  out[i,:] = (sum_j e0[j] XV[j,:] + delta[i] XV[i,:]) / (S - e0[i] + w1[i]) + V_b
XV is computed early (only needs xh and V_w), so after the scalar softmax
pieces the output is just two fused DVE ops.
(The max-subtraction is dropped: softmax is shift-invariant and the logits for
this problem are O(13), far from f32 exp overflow.)

LayerNorm affine (ln_w, ln_b) is folded into enc_w/enc_b/c0/dec_w on the host;
z_hat = (z - mu) * rstd is the only on-chip normalization, and the rstd scale
rides the z-transpose as a diagonal right-operand (transpose out = zc^T @
diag(rstd)).

Activation-table plan: Gelu (z half) -> Sqrt -> everything else (tanh + exp
live in the same exp_and_others set): xh-gelu and the AE-hidden gelu use the
tanh approximation (cubic on DVE, Tanh on ACT), so only two mid-kernel table
loads occur and both overlap compute.

Sharding: 8 cores = (4 batches) x (2 halves of the query dim). Token order is
rolled per-core on the host (attention here is permutation-equivariant), so each
core always computes output rows 0:64 of its (rolled) batch.

DMA: the z-half of U_w ships with x in the first load so the z matmul starts
as early as possible; triggers are split across the two HWDGE queues (SP+ACT).
"""
import math
import sys

if '/opt/trn_rl_repo' not in sys.path:
    sys.path.insert(0, '/opt/trn_rl_repo')

import numpy as np

B, N, FEAT, FFN, ZDIM, HID = 4, 128, 384, 768, 384, 192
LN_EPS = 1e-5
NCORES = 8
OWN = 64  # output rows per core

COMPUTE_DT = "bf16"

# gelu_tanh(x) = t*(1+tanh(GK1*t + GK3*t^3)) with t = x/2
GK1 = 2.0 * 0.7978845608028654
GK3 = 8.0 * 0.044715 * 0.7978845608028654

# kaux columns (compute dtype; matmul operands only):
#   0:128 identity | 128:256 ones | 256:260 c0' (3 cols + pad) |
#   260 dlnb[0:128] | 261 dlnb[128:192] (rows 0:64) | 262:272 pad
KAUX_COLS = 272
# auxf columns (f32): 0 enc_b'[0:128] | 1 enc_b'[128:192] (rows 0:64) |
#   2:5 dec_b' | 5 d0 const | 6 d1 const (rows 0:64) | 7 pad |
#   8:392 V_b broadcast (rows 0:64)
AUXF_COLS = 8 + FEAT

_CACHE = {}


def _patch_tile_drain(tile):
    """walrus in this container only accepts 1 sync-wait command per CTRL
    instruction; Tile's kernel-tail drain can carry many. Split the drain's
    waits over several drain instructions."""
    if getattr(tile.TileContext, '_drain_patched', False):
        return
    from concourse import mybir

    def _drain_and_barrier(self, tick_clock, wait_clock):
        nc = self.nc
        drain_inst = nc.sync.drain()
        wait_clock.add_sem_waits(
            drain_inst.ins, tile.ScopedClock({None: tick_clock.global_clock})
        )
        mi = drain_inst.ins
        waits = list(mi.sync_info.on_wait) if mi.sync_info else []
        if len(waits) > 1:
            mi.sync_info = mybir.SyncInfo(on_wait=waits[:1], on_update=[])
            engines = [nc.sync, nc.gpsimd, nc.scalar, nc.vector, nc.tensor]
            for i, wt_ in enumerate(waits[1:]):
                n2 = engines[i % len(engines)].nop()
                n2.ins.sync_info = mybir.SyncInfo(on_wait=[wt_], on_update=[])
        nc.all_engine_barrier()
        assert self.sems is not None
        popped = self.nc._tile_sem_poison_stack.pop()
        assert popped is self._sem_poison
        nc.clear_and_free_semaphores(list(self.sems.allocated().values()))
        nc.all_engine_barrier()

    tile.TileContext._drain_and_barrier = _drain_and_barrier
    tile.TileContext._drain_patched = True


def _split_excess_waits(nc, mybir, maxw=1):
    """This container's walrus accepts only one sync-wait command per
    instruction. Move excess waits onto InstNoOp carriers inserted just before
    the over-subscribed instruction on the same engine."""
    for fn in nc.m.functions:
        for blk in fn.blocks:
            new = []
            changed = False
            for inst in blk.instructions:
                si = inst.sync_info
                waits = list(si.on_wait) if si and si.on_wait else []
                if len(waits) > maxw:
                    changed = True
                    extra = waits[:-maxw]
                    ups = list(si.on_update) if si.on_update else []
                    inst.sync_info = mybir.SyncInfo(
                        on_wait=waits[-maxw:], on_update=ups)
                    for i in range(0, len(extra), maxw):
                        nop = mybir.InstNoOp(
                            name=nc.get_next_instruction_name(),
                            engine=inst.engine, ins=[], outs=[])
                        nop.sync_info = mybir.SyncInfo(
                            on_wait=extra[i:i + maxw], on_update=[])
                        new.append(nop)
                new.append(inst)
            if changed:
                blk.instructions = new


def _build_nc(dt_name):
    import concourse.bass as bass
    import concourse.tile as tile
    from concourse import mybir

    _patch_tile_drain(tile)

    F32 = mybir.dt.float32
    DT = {"bf16": mybir.dt.bfloat16, "f32r": mybir.dt.float32r,
          "f32": mybir.dt.float32}[dt_name]
    AF = mybir.ActivationFunctionType
    OP = mybir.AluOpType
    AX = mybir.AxisListType

    nc = bass.Bass()
    dp = nc.declare_dram_parameter
    d_xuwz = dp("xuwz", [128, 384 + 3 * 384], DT, isOutput=False)
    d_uwxh = dp("uwxh", [128, 3 * 384], DT, isOutput=False)
    d_kaux = dp("kaux", [128, KAUX_COLS], DT, isOutput=False)
    d_auxf = dp("auxf", [128, AUXF_COLS], F32, isOutput=False)
    d_ubvb = dp("ubvb", [1, 128 + FFN + FEAT], DT, isOutput=False)
    d_bigw = dp("bigw", [128, 3 * HID + 2 * ZDIM + 3 * FEAT], DT,
                isOutput=False)
    d_out = dp("out", [OWN, FEAT], F32, isOutput=True)

    mm = nc.tensor.matmul

    with tile.TileContext(nc) as tc:
        with tc.tile_pool(name="w", bufs=1) as w, \
             tc.tile_pool(name="ps", bufs=1, space="PSUM") as ps:

            def wt(name, p, f, dt=None):
                return w.tile([p, f], dt or DT, name=name, tag=name)

            # ---- input DMAs, split across the two HWDGE queues ----
            xuwz = wt("xuwz", 128, 384 + 3 * 384)
            nc.sync.dma_start(xuwz[:, :], d_xuwz[:, :])
            ubvb = wt("ubvb", 1, 128 + FFN + FEAT)
            nc.scalar.dma_start(ubvb[:, :], d_ubvb[:, :])
            uwxh = wt("uwxh", 128, 3 * 384)
            nc.sync.dma_start(uwxh[:, :], d_uwxh[:, :])
            kaux = wt("kaux", 128, KAUX_COLS)
            nc.scalar.dma_start(kaux[:, :], d_kaux[:, :])
            auxf = wt("auxf", 128, AUXF_COLS, F32)
            nc.scalar.dma_start(auxf[:, :], d_auxf[:, :])
            bigw = wt("bigw", 128, 3 * HID + 2 * ZDIM + 3 * FEAT)
            nc.scalar.dma_start(bigw[:, :], d_bigw[:, :])

            # ---- views ----
            ident = kaux[:, 0:128]
            ones2 = kaux[:, 128:130]
            ones128 = kaux[:, 128:256]
            ones_row = ubvb[0:1, 0:128]
            c0p = kaux[:, 256:260]
            dlnb0 = kaux[:, 260:262]
            dlnb1 = kaux[0:OWN, 261:263]
            encb0 = auxf[:, 0:1]
            encb1 = auxf[0:OWN, 1:2]
            decb = auxf[:, 2:5]
            d0cb = auxf[:, 5:6]
            dblb = auxf[0:OWN, 6:7]
            vbbc = auxf[0:OWN, 8:8 + FEAT]
            ub = ubvb[0:1, 128:128 + FFN]
            encw = bigw[:, 0:576]
            decw = bigw[:, 576:1344]
            vw = bigw[:, 1344:2496]
            xT = xuwz[:, 0:384]

            # ---- U matmul, z half first (bias rides first) ----
            ps_z = ps.tile([128, 384], F32, name="ps_z", tag="big", bufs=2)
            mm(ps_z[:, :], ones_row[0:1, 0:128], ub[0:1, 384:768],
               start=True, stop=False)
            for k in range(3):
                mm(ps_z[:, :], xT[:, 128 * k:128 * (k + 1)],
                   xuwz[:, 384 * (k + 1):384 * (k + 2)],
                   start=False, stop=(k == 2))
            z0 = wt("z0", 128, 384)
            nc.scalar.activation(z0[:, :], ps_z[:, :], AF.Gelu)

            ps_xh = ps.tile([128, 384], F32, name="ps_xh", tag="big", bufs=2)
            mm(ps_xh[:, :], ones_row[0:1, 0:128], ub[0:1, 0:384],
               start=True, stop=False)
            for k in range(3):
                mm(ps_xh[:, :], xT[:, 128 * k:128 * (k + 1)],
                   uwxh[:, 384 * k:384 * (k + 1)],
                   start=False, stop=(k == 2))

            # ---- LayerNorm stats (one pass) + centered z ----
            epsc = wt("epsc", 128, 1, F32)
            nc.vector.memset(epsc[:, :], LN_EPS)
            musum = wt("musum", 128, 1, F32)
            nc.vector.reduce_sum(musum[:, :], z0[:, :], axis=AX.X)
            sq = wt("sq", 128, 384)
            vsum = wt("vsum", 128, 1, F32)
            nc.vector.scalar_tensor_tensor(sq[:, :], z0[:, :], 1.0, z0[:, :],
                                           op0=OP.mult, op1=OP.mult,
                                           accum_out=vsum[:, 0:1])
            musq = wt("musq", 128, 1, F32)
            nc.vector.scalar_tensor_tensor(musq[:, :], musum[:, :], 1.0 / ZDIM,
                                           musum[:, :], op0=OP.mult,
                                           op1=OP.mult)
            v2 = wt("v2", 128, 1, F32)
            nc.vector.tensor_sub(v2[:, :], vsum[:, :], musq[:, :])
            negmu = wt("negmu", 128, 1, F32)
            nc.vector.tensor_scalar_mul(negmu[:, :], musum[:, :], -1.0 / ZDIM)
            zc = wt("zc", 128, 384)
            nc.vector.tensor_scalar_add(zc[:, :], z0[:, :], negmu[:, 0:1])
            std = wt("std", 128, 1, F32)
            nc.scalar.activation(std[:, :], v2[:, :], AF.Sqrt,
                                 bias=epsc[:, 0:1], scale=1.0 / ZDIM)
            rstd = wt("rstd", 128, 1, F32)
            nc.vector.reciprocal(rstd[:, :], std[:, :])
            # z_hat = zc * rstd (per-token scale; zc was centered early)
            zn = wt("zn", 128, 384)
            nc.vector.tensor_scalar(zn[:, :], zc[:, :], rstd[:, 0:1],
                                    None, op0=OP.mult)

            # ---- xh gelu via tanh approx ----
            # bypass-read of `sq` pins this chain after the LN stats on DVE
            # (the list scheduler otherwise hoists it into the stats window).
            xt1 = wt("xt1", 128, 384)
            nc.vector.scalar_tensor_tensor(xt1[:, :], ps_xh[:, :], 0.5,
                                           sq[:, :], op0=OP.mult,
                                           op1=OP.bypass)
            xq = wt("xq", 128, 384)
            nc.vector.scalar_tensor_tensor(xq[:, :], xt1[:, :], GK3,
                                           xt1[:, :], op0=OP.mult,
                                           op1=OP.mult)
            xin = wt("xin", 128, 384)
            nc.vector.scalar_tensor_tensor(xin[:, :], xq[:, :], GK1,
                                           xt1[:, :],
                                           op0=OP.add, op1=OP.mult)
            xth = wt("xth", 128, 384)
            nc.scalar.activation(xth[:, :], xin[:, :], AF.Tanh)
            xh = wt("xh", 128, 384)
            nc.vector.scalar_tensor_tensor(xh[:, :], xth[:, :], 1.0,
                                           xt1[:, :], op0=OP.add,
                                           op1=OP.mult)

            # ---- transpose z_hat -> zT [feat, token] ----
            zT = wt("zT", 128, 384)
            for k in range(3):
                pt = ps.tile([128, 128], DT, name="pt", tag="pt", bufs=2)
                nc.tensor.transpose(pt[:, :], zn[:, 128 * k:128 * (k + 1)],
                                    ident)
                nc.vector.tensor_copy(zT[:, 128 * k:128 * (k + 1)], pt[:, :])

            # ---- logit column d0 ----
            ps_d0c = ps.tile([128, 2], F32, name="ps_d0c", tag="sm2", bufs=2)
            for k in range(3):
                mm(ps_d0c[:, :], zT[:, 128 * k:128 * (k + 1)],
                   c0p[:, k:k + 2], start=(k == 0), stop=(k == 2))

            # ---- AE enc ----
            ps_h0 = ps.tile([128, OWN], F32, name="ps_h0", tag="sm", bufs=2)
            ps_h1 = ps.tile([OWN, OWN], F32, name="ps_h1", tag="sm", bufs=2)
            for k in range(3):
                rhs = zT[:, 128 * k:128 * k + OWN]
                mm(ps_h0[:, :], encw[:, 192 * k:192 * k + 128], rhs,
                   start=(k == 0), stop=(k == 2))
                mm(ps_h1[:, :], encw[:, 192 * k + 128:192 * (k + 1)], rhs,
                   start=(k == 0), stop=(k == 2))

            # ---- AE hidden gelu (tanh approx; cubic on DVE, Tanh on ACT) ----
            def tg_inner(psum, bias_col, p, name):
                t1 = wt(name + "_t1", p, OWN)
                nc.vector.tensor_scalar(t1[:, :], psum[:, :], bias_col, 0.5,
                                        op0=OP.add, op1=OP.mult)
                q = wt(name + "_q", p, OWN)
                nc.vector.scalar_tensor_tensor(q[:, :], t1[:, :], GK3,
                                               t1[:, :], op0=OP.mult,
                                               op1=OP.mult)
                inner = wt(name + "_in", p, OWN)
                nc.vector.scalar_tensor_tensor(inner[:, :], q[:, :],
                                               GK1, t1[:, :],
                                               op0=OP.add, op1=OP.mult)
                return t1, inner

            t1_0, in_0 = tg_inner(ps_h0, encb0[:, 0:1], 128, "h0")
            t1_1, in_1 = tg_inner(ps_h1, encb1[:, 0:1], OWN, "h1")
            th0 = wt("th0", 128, OWN)
            nc.scalar.activation(th0[:, :], in_0[:, :], AF.Tanh)
            th1 = wt("th1", OWN, OWN)
            nc.scalar.activation(th1[:, :], in_1[:, :], AF.Tanh)
            h0 = wt("h0", 128, OWN)
            nc.vector.scalar_tensor_tensor(h0[:, :], th0[:, :], 1.0,
                                           t1_0[:, :], op0=OP.add,
                                           op1=OP.mult)
            h1 = wt("h1", OWN, OWN)
            nc.vector.scalar_tensor_tensor(h1[:, :], th1[:, :], 1.0,
                                           t1_1[:, :], op0=OP.add,
                                           op1=OP.mult)

            # ---- e0 = exp(d0 + d0const) (slack; after tanh ops on ACT) ----
            e0 = wt("e0", 128, 2)
            nc.gpsimd.tensor_copy(e0[:, 1:2], ones2[:, 0:1])
            nc.scalar.activation(e0[:, 0:1], ps_d0c[:, 0:1], AF.Exp,
                                 bias=d0cb[:, 0:1])

            # ---- slack pipeline: xh^T, XV = xh @ V_w, e0-weighted sum ----
            xhT = wt("xhT", 128, 384)
            for k in range(3):
                pt2 = ps.tile([128, 128], DT, name="pt2", tag="pt", bufs=2)
                nc.tensor.transpose(pt2[:, :], xh[:, 128 * k:128 * (k + 1)],
                                    ident)
                nc.vector.tensor_copy(xhT[:, 128 * k:128 * (k + 1)],
                                      pt2[:, :])
            ps_xv = ps.tile([128, 384], F32, name="ps_xv", tag="big", bufs=2)
            for k in range(3):
                mm(ps_xv[:, :], xhT[:, 128 * k:128 * (k + 1)],
                   vw[:, 384 * k:384 * (k + 1)], start=(k == 0), stop=(k == 2))
            xv = wt("xv", 128, 384)
            nc.vector.tensor_copy(xv[:, :], ps_xv[:, :])
            e0bc = wt("e0bc", 128, OWN)
            nc.gpsimd.tensor_copy(e0bc[:, :],
                                  e0[:, 0:1].broadcast_to([128, OWN]))
            ps_tbv = ps.tile([OWN, 384], F32, name="ps_tbv", tag="big",
                             bufs=2)
            mm(ps_tbv[:, :], e0bc[:, :], xv[:, :], start=True, stop=True)
            ps_sbc = ps.tile([128, 2], F32, name="ps_sbc", tag="sm2", bufs=2)
            mm(ps_sbc[:, :], ones128, e0[:, 0:2], start=True, stop=True)

            # ---- AE dec + P = (dec_out + dec_b') * z_hat ----
            Pt = wt("Pt", 128, 3 * OWN)
            for k in range(3):
                ps_d = ps.tile([128, OWN], F32, name="ps_d", tag="pt", bufs=2)
                mm(ps_d[:, :], decw[0:128, 128 * k:128 * (k + 1)], h0[:, :],
                   start=True, stop=False)
                mm(ps_d[:, :], decw[0:OWN, 384 + 128 * k:384 + 128 * (k + 1)],
                   h1[:, :], start=False, stop=True)
                nc.vector.scalar_tensor_tensor(
                    Pt[:, OWN * k:OWN * (k + 1)], ps_d[:, :], decb[:, k:k + 1],
                    zT[:, 128 * k:128 * k + OWN], op0=OP.add, op1=OP.mult)

            # ---- diag logit d1 ----
            ps_d1c = ps.tile([OWN, 2], F32, name="ps_d1c", tag="sm2", bufs=2)
            for k in range(3):
                mm(ps_d1c[:, :], Pt[:, OWN * k:OWN * (k + 1)],
                   ones2, start=(k == 0), stop=False)
            mm(ps_d1c[:, :], h0[:, :], dlnb0, start=False, stop=False)
            mm(ps_d1c[:, :], h1[:, :], dlnb1, start=False, stop=True)
            w1 = wt("w1", OWN, 1, F32)
            nc.scalar.activation(w1[:, 0:1], ps_d1c[:, 0:1], AF.Exp,
                                 bias=dblb[:, 0:1])

            # ---- softmax scalars ----
            delta = wt("delta", OWN, 1, F32)
            nc.vector.tensor_sub(delta[:, :], w1[:, :], e0[0:OWN, 0:1])
            rden = wt("rden", OWN, 1, F32)
            denom = wt("denom", OWN, 1, F32)

            # ---- epilogue: res = rden*TBV + (delta*rden)*XV_own + vb ----
            nc.vector.tensor_add(denom[:, :], delta[:, :], ps_sbc[0:OWN, 0:1])
            nc.vector.reciprocal(rden[:, :], denom[:, :])
            drd = wt("drd", OWN, 1, F32)
            nc.vector.scalar_tensor_tensor(drd[:, :], delta[:, :], 1.0,
                                           rden[:, :], op0=OP.mult,
                                           op1=OP.mult)
            tmid = wt("tmid", OWN, 384, F32)
            nc.vector.scalar_tensor_tensor(tmid[:, :], xv[0:OWN, :],
                                           drd[:, 0:1], vbbc[:, :],
                                           op0=OP.mult, op1=OP.add)
            res = wt("res", OWN, 384, F32)
            nc.vector.scalar_tensor_tensor(res[:, :], ps_tbv[:, :],
                                           rden[:, 0:1], tmid[:, :],
                                           op0=OP.mult, op1=OP.add)
            nc.sync.dma_start(d_out[:, :], res[:, :])

    _split_excess_waits(nc, mybir)
    return nc


def _gelu64(x):
    x = np.asarray(x, dtype=np.float64)
    erf = np.vectorize(math.erf)
    return x * 0.5 * (1.0 + erf(x / math.sqrt(2.0)))


def _np_dt(dt_name):
    if dt_name == "bf16":
        import ml_dtypes
        return ml_dtypes.bfloat16
    return np.float32


def _prep_weights(U_w, U_b, ln_w, ln_b, enc_w, enc_b, dec_w, dec_b, V_w, V_b,
                  dt_name=None):
    dt_name = dt_name or COMPUTE_DT
    ndt = _np_dt(dt_name)
    f32 = lambda a: np.ascontiguousarray(np.asarray(a, dtype=np.float32))
    cvt = lambda a: np.ascontiguousarray(np.asarray(a).astype(ndt))
    lnw, lnb = f32(ln_w), f32(ln_b)

    # uw chunks: chunk k = U_w[128k:128(k+1), :]; z half = cols 384:768
    uw = f32(U_w).reshape(3, 128, FFN).transpose(1, 0, 2)  # [128, 3, 768]
    uwz = uw[:, :, 384:768].reshape(128, 3 * 384)
    uwxh = uw[:, :, 0:384].reshape(128, 3 * 384)

    encw_f = lnw[:, None] * f32(enc_w)
    encb_f = f32(enc_b) + lnb @ f32(enc_w)
    decw_f = f32(dec_w) * lnw[None, :]
    decb_f = f32(dec_b) * lnw
    dlnb = f32(dec_w) @ lnb
    dbl = float(f32(dec_b) @ lnb)
    c0 = (_gelu64(enc_b) @ np.asarray(dec_w, np.float64)
          + np.asarray(dec_b, np.float64)).astype(np.float32)
    c0p = c0 * lnw
    d0c = float(c0 @ lnb)

    encw = encw_f.reshape(3, 128, HID).transpose(1, 0, 2).reshape(128, 3 * HID)
    vwf = f32(V_w).reshape(3, 128, FEAT).transpose(1, 0, 2).reshape(128, 3 * FEAT)
    decw = np.zeros((128, 2 * ZDIM), np.float32)
    decw[:, :ZDIM] = decw_f[0:128, :]
    decw[:OWN, ZDIM:] = decw_f[128:192, :]
    bigw = np.concatenate([encw, decw, vwf], axis=1)

    kaux = np.zeros((128, KAUX_COLS), np.float32)
    kaux[:, 0:128] = np.eye(128, dtype=np.float32)
    kaux[:, 128:256] = 1.0
    kaux[:, 256:259] = c0p.reshape(3, 128).T
    kaux[:, 260] = dlnb[0:128]
    kaux[:OWN, 261] = dlnb[128:192]

    auxf = np.zeros((128, AUXF_COLS), np.float32)
    auxf[:, 0] = encb_f[0:128]
    auxf[:OWN, 1] = encb_f[128:192]
    auxf[:, 2:5] = decb_f.reshape(3, 128).T
    auxf[:, 5] = d0c
    auxf[:OWN, 6] = dbl
    auxf[:OWN, 8:8 + FEAT] = f32(V_b)[None, :]

    ubvb = np.concatenate([np.ones(128, np.float32), f32(U_b),
                           f32(V_b)]).reshape(1, 128 + FFN + FEAT)
    return {
        "uwz": cvt(uwz),
        "uwxh": cvt(uwxh),
        "kaux": cvt(kaux),
        "auxf": auxf,
        "ubvb": cvt(ubvb),
        "bigw": cvt(bigw),
    }


def _get_nc(dt_name=None):
    dt_name = dt_name or COMPUTE_DT
    key = ("nc", dt_name)
    if key not in _CACHE:
        _CACHE[key] = _build_nc(dt_name)
    return _CACHE[key]


def make_in_maps(x, weights, dt_name=None):
    dt_name = dt_name or COMPUTE_DT
    ndt = _np_dt(dt_name)
    x = np.asarray(x).astype(ndt)
    shared = {k: weights[k] for k in ("uwxh", "kaux", "auxf", "ubvb", "bigw")}
    in_maps = []
    for c in range(NCORES):
        b, ih = divmod(c, 2)
        xs = np.roll(x[b], -OWN * ih, axis=0)
        # pre-transpose: [128, 3*128] where chunk k = x[:, 128k:128(k+1)].T
        xt = xs.T.reshape(3, 128, 128).transpose(1, 0, 2).reshape(128, 384)
        xuwz = np.ascontiguousarray(
            np.concatenate([xt, weights["uwz"]], axis=1))
        in_maps.append({"xuwz": xuwz, **shared})
    return in_maps


def assemble(results):
    out = np.empty((B, N, FEAT), np.float32)
    for c in range(NCORES):
        b, ih = divmod(c, 2)
        out[b, OWN * ih:OWN * (ih + 1), :] = results[c]["out"]
    return out


def kernel(x, U_w, U_b, ln_w, ln_b, enc_w, enc_b, dec_w, dec_b, V_w, V_b):
    from concourse.bass_utils import run_bass_kernel_spmd
    nc = _get_nc()
    weights = _prep_weights(U_w, U_b, ln_w, ln_b, enc_w, enc_b, dec_w, dec_b,
                            V_w, V_b)
    in_maps = make_in_maps(x, weights)
    r = run_bass_kernel_spmd(nc, in_maps, core_ids=list(range(NCORES)))
    return assemble(r.results)
